# revision 14
# baseline (speedup 1.0000x reference)
"""Cumulative-min along time for trace[16, 8192, 256] on 8 TRN2 NeuronCores.

Data-parallel sharding (no collectives): batch dim 16 -> 2 per core.

The host prepares each core's shard in feature-major layout [2, 256, 8192]
(time contiguous), so on-device the cumulative min is a pure streaming
workload: DMA a [128 lanes, TT time] tile in, run the DVE hardware prefix
scan (tensor_tensor_scan with op=min) along the free dim, DMA out.
Carries chain consecutive time chunks per lane. No tensor-engine work,
no PSUM: the kernel runs at the HBM roofline. The host transposes the
result back to [b, t, f] while gathering.
"""

import sys
import types

import numpy as np

import concourse.bass as bass
import concourse.tile as tile
from concourse import bacc, mybir
from concourse.bass_utils import run_bass_kernel_spmd


def _ensure_profile_hook():
    """If the image's antenv package lacks axon_hooks (as in this
    container), NTFF profiling under BASS_TRACE=1 would crash on import.
    Provide the hook via trn_agent_boot's ctypes fallback and make
    artifact upload degrade gracefully. No-op when the real module
    exists."""
    try:
        import antenv.axon_hooks  # noqa: F401
        return
    except ImportError:
        pass
    try:
        import trn_agent_boot.trn_boot as tb
        import concourse.bass_utils as bu

        hook = tb._ntff_profile_via_ctypes("/opt/axon/libaxon_pjrt.so")
        mod = types.ModuleType("antenv.axon_hooks")
        mod.get_axon_ntff_profile_hook = lambda: hook
        mod.set_axon_ntff_profile_hook = lambda h: None
        sys.modules["antenv.axon_hooks"] = mod

        orig_upload = bu.upload_artifacts

        def _safe_upload(tmpdir):
            try:
                return orig_upload(tmpdir)
            except Exception:
                return f"file://{tmpdir}"

        bu.upload_artifacts = _safe_upload
    except Exception:
        pass


_ensure_profile_hook()

N_CORES = 8
B, T, F = 16, 8192, 256
B_LOC = B // N_CORES  # batches per core

P = 128          # partitions (lanes per tile)
TT = 2048        # time steps per scan chunk
BIG = 3.0e38     # scan init: min(x, BIG) == x for all finite f32 inputs

F32 = mybir.dt.float32


def build_program(b_loc=B_LOC, t=T, f=F, tt=TT):
    lanes = b_loc * f
    n_lt = lanes // P        # lane tiles
    n_c = t // tt            # time chunks per lane
    nc = bacc.Bacc("TRN2", target_bir_lowering=False, debug=False)
    x = nc.dram_tensor("trace", [lanes, t], F32, kind="ExternalInput").ap()
    y = nc.dram_tensor("out", [lanes, t], F32, kind="ExternalOutput").ap()

    with tile.TileContext(nc) as tc:
        with (
            tc.tile_pool(name="ld", bufs=8) as ld_pool,
            # res tiles double as carry sources for the next chunk of the
            # same lane tile, so one slot per concurrently-live chain plus
            # slack for store overlap.
            tc.tile_pool(name="res", bufs=12) as res_pool,
        ):
            carries = [None] * n_lt
            for c in range(n_c):
                for lt in range(n_lt):
                    ld = ld_pool.tile([P, tt], F32)
                    nc.sync.dma_start(
                        out=ld[:],
                        in_=x[lt * P:(lt + 1) * P, c * tt:(c + 1) * tt],
                    )
                    res = res_pool.tile([P, tt], F32)
                    init = carries[lt] if carries[lt] is not None else BIG
                    nc.vector.tensor_tensor_scan(
                        out=res[:],
                        data0=ld[:],
                        data1=ld[:],  # ignored by op1=bypass
                        initial=init,
                        op0=mybir.AluOpType.min,
                        op1=mybir.AluOpType.bypass,
                    )
                    carries[lt] = res[:, tt - 1:tt]
                    # stores issue from the Activation HWDGE queue so a
                    # store blocked on its scan can't head-of-line block
                    # load issue on the Sync sequencer
                    nc.scalar.dma_start(
                        out=y[lt * P:(lt + 1) * P, c * tt:(c + 1) * tt],
                        in_=res[:],
                    )

    nc.compile()
    return nc


_PROG = None


def _get_prog():
    global _PROG
    if _PROG is None:
        _PROG = build_program()
    return _PROG


def run(in_maps, **kwargs):
    nc = _get_prog()
    return run_bass_kernel_spmd(nc, in_maps, core_ids=list(range(N_CORES)), **kwargs)


def make_in_maps(trace):
    trace = np.asarray(trace, dtype=np.float32)
    maps = []
    for i in range(N_CORES):
        shard = trace[i * B_LOC:(i + 1) * B_LOC]          # [2, T, F]
        shard = np.ascontiguousarray(shard.transpose(0, 2, 1))  # [2, F, T]
        maps.append({"trace": shard.reshape(B_LOC * F, T)})
    return maps


def kernel(trace):
    res = run(make_in_maps(trace))
    parts = []
    for i in range(N_CORES):
        o = res.results[i]["out"].reshape(B_LOC, F, T)
        parts.append(o.transpose(0, 2, 1))                # [2, T, F]
    return np.ascontiguousarray(np.concatenate(parts, axis=0))
